# revision 1
# baseline (speedup 1.0000x reference)
"""2-layer GCN (GCNConv x2 + log_softmax) on 8 Trainium2 NeuronCores.

Algorithm
---------
out = log_softmax( A_hat @ relu(A_hat @ (x@W1) + b1) @ W2 + b2 )
with A_hat = D^-1/2 (A + I) D^-1/2 (in-degree + self-loop).

Structure:
  * norm(s,d) = dis[s]*dis[d] is separable: dis[src] is folded into the
    gathered table rows (h_hat = dis * h), dis[dst] into the output ->
    no per-edge float arithmetic at all.
  * Both propagations run at width 16: A_hat(zW2) == (A_hat z)W2.
  * Nodes sharded in contiguous ranges of 12500 per core (edges live with
    their dst owner).  Each core computes h_hat for its own rows; an
    AllGather replicates the full table into every core's HBM; per-edge
    gathers + segment sums run locally.
  * Gathers use the TIE-accelerated InstDMAGatherAnt (dma_gather): 256B
    rows (16 payload floats + 48 pad), int16 indices.  The int16 range
    forces 4 table chunks of 25088 rows (2 cores each).
  * Scatter-free segment sum per chunk: dst nodes sorted by per-chunk
    in-degree ("rank_q" order); round r gathers the r-th incoming message
    for the first n_r ranks -- a contiguous accumulator prefix -- so
    accumulation is a plain dense DVE add (dma_gather lands row i at
    partition i%128, slot i//128, exactly the accumulator layout).
  * The 4 per-chunk accumulators are merged back to node order with 4 more
    dma_gathers (from HBM dumps); the self-loop term is the locally-held
    table row, a plain add.  Everything downstream is node-ordered, so
    layer 1 and layer 2 share identical gather index arrays.
  * SPMD: one program for all 8 cores; per-(chunk,round) sizes are maxed
    across cores, shortfall entries point at a zeroed dead-slot row.
"""

import os
import sys
from contextlib import ExitStack

import numpy as np

if "/opt/trn_rl_repo" not in sys.path:
    sys.path.insert(0, "/opt/trn_rl_repo")

# ---------------------------------------------------------------- constants
N_NODES = 100000
NCORES = 8
F_IN = 512
HID = 16
NCLS = 40
P = 128
EP = 64          # table row pitch in f32 elems (256B, dma_gather minimum)
NCHUNK = 4       # table chunks (2 cores each) to fit int16 indices
SUBCAP = 8       # slots per dma_gather (1024 idxs: single_packet-safe)

LAST_EXEC_NS = None


def _dims():
    nloc = N_NODES // NCORES
    tslot = -(-nloc // P)          # ceil -> accumulator slots
    return nloc, tslot, tslot * P


def _wrap16(flat):
    """int16 index vector -> [128, n/16] dma_gather layout (16-partition
    wrap, replicated 8x down the partitions)."""
    n = flat.size
    assert n % 16 == 0
    blk = flat.reshape(n // 16, 16).T
    return np.tile(blk, (8, 1)).astype(np.int16)


# ================================================================ host plan
def _plan(edge_index):
    nloc, tslot, padloc = _dims()
    cpc = NCORES // NCHUNK              # cores per chunk
    crows = cpc * padloc                # table rows per chunk
    assert crows - 1 <= np.iinfo(np.int16).max
    pad_row = padloc - 1          # dead (zeroed) row of chunk's 1st core
    assert padloc > nloc

    src = np.asarray(edge_index[0]).astype(np.int64)
    dst = np.asarray(edge_index[1]).astype(np.int64)
    owner = dst // nloc

    # chunk-local table row of global node g  (within chunk g // (cpc*nloc));
    # table dumps are node-id-major: row = rel_core * padloc + local_node
    def chunk_row(g):
        return ((g // nloc) % cpc) * padloc + g % nloc

    # per (core, chunk): degree-ranked rounds
    entries = [[None] * NCHUNK for _ in range(NCORES)]   # per-round row lists
    nrounds = np.zeros((NCORES, NCHUNK), np.int64)
    merge_idx = np.empty((NCORES, NCHUNK, P, padloc // 16), np.int16)
    deg_node = np.full((NCORES, P, tslot), 1e38, np.float32)

    for c in range(NCORES):
        m = owner == c
        s_c = src[m]
        l_c = dst[m] - c * nloc
        deg_tot = np.bincount(l_c, minlength=nloc)
        dn = np.full(padloc, 1e38, np.float32)
        dn[:nloc] = deg_tot + 1.0
        deg_node[c] = dn.reshape(tslot, P).T

        cq = s_c // (cpc * nloc)
        for q in range(NCHUNK):
            mq = cq == q
            s_q, l_q = s_c[mq], l_c[mq]
            deg_q = np.bincount(l_q, minlength=nloc)
            order_q = np.argsort(-deg_q, kind="stable")
            rank_of = np.empty(nloc, np.int64)
            rank_of[order_q] = np.arange(nloc)
            r_e = rank_of[l_q]
            o1 = np.argsort(r_e, kind="stable")
            rs, ss = r_e[o1], s_q[o1]
            deg_rank = deg_q[order_q]
            starts = np.zeros(nloc, np.int64)
            starts[1:] = np.cumsum(deg_rank)[:-1]
            occ = np.arange(rs.size, dtype=np.int64) - starts[rs]
            o2 = np.argsort(occ * nloc + rs, kind="stable")
            rows_sorted = chunk_row(ss[o2]).astype(np.int16)
            n_r = (np.bincount(occ).astype(np.int64) if occ.size
                   else np.zeros(0, np.int64))
            offs = np.zeros(n_r.size + 1, np.int64)
            offs[1:] = np.cumsum(n_r)
            entries[c][q] = (rows_sorted, n_r, offs)
            nrounds[c, q] = n_r.size

            mi = np.full(padloc, padloc - 1, np.int64)  # dead -> zeroed row
            mi[:nloc] = rank_of
            merge_idx[c, q] = _wrap16(mi.astype(np.int16))

    # unified round sizes (max over cores), then greedy batches per chunk
    rounds_q = []
    for q in range(NCHUNK):
        rmax = int(nrounds[:, q].max())
        nm = np.zeros(rmax, np.int64)
        for c in range(NCORES):
            n_r = entries[c][q][1]
            nm[: n_r.size] = np.maximum(nm[: n_r.size], n_r)
        rounds_q.append([int(v) for v in nm if v > 0])

    s_slots = tslot                       # staging = one accumulator width
    batches = []                          # (chunk, [(slot_offset, n), ...])
    for q in range(NCHUNK):
        cur, cur_slots = [], 0
        for n in rounds_q[q]:
            k = -(-n // P)
            if cur_slots + k > s_slots:
                batches.append((q, cur))
                cur, cur_slots = [], 0
            cur.append((cur_slots, n))
            cur_slots += k
        if cur:
            batches.append((q, cur))
    ng = len(batches)

    gidx = np.full((NCORES, ng, P, s_slots * 8), pad_row, np.int16)
    for c in range(NCORES):
        rpos = [0] * NCHUNK
        for b, (q, rl) in enumerate(batches):
            rows_sorted, n_r, offs = entries[c][q]
            flat = np.full(s_slots * P, pad_row, np.int64)
            for (o, n) in rl:
                r = rpos[q]
                if r < n_r.size:
                    v = rows_sorted[offs[r]: offs[r + 1]]
                    flat[o * P: o * P + v.size] = v
                rpos[q] += 1
            gidx[c, b] = _wrap16(flat.astype(np.int16))

    meta = dict(batches=batches, ng=ng, s_slots=s_slots,
                tslot=tslot, nloc=nloc, padloc=padloc, crows=crows)
    host = dict(gidx=gidx, midx=merge_idx, deg_node=deg_node)
    return meta, host


# ============================================================ device program
def _emit(tc, io, meta, reps=1):
    import concourse.tile as tile  # noqa: F401
    from concourse import mybir

    nc = tc.nc
    f32 = mybir.dt.float32
    i16 = mybir.dt.int16
    nloc, tslot, padloc = meta["nloc"], meta["tslot"], meta["padloc"]
    s_slots, crows = meta["s_slots"], meta["crows"]
    batches, ng = meta["batches"], meta["ng"]
    kch = F_IN // P
    AF = mybir.ActivationFunctionType
    AL = mybir.AluOpType
    ABLATE = os.environ.get("BASS_GCN_ABLATE", "")

    with ExitStack() as ctx:
        sb = ctx.enter_context(tc.tile_pool(name="sb", bufs=1))
        xb = ctx.enter_context(tc.tile_pool(name="xb", bufs=3))
        stg = ctx.enter_context(tc.tile_pool(name="stg", bufs=3))
        ib = ctx.enter_context(tc.tile_pool(name="ib", bufs=3))
        tp = ctx.enter_context(tc.tile_pool(name="tp", bufs=2))
        ps = ctx.enter_context(tc.tile_pool(name="ps", bufs=2, space="PSUM"))
        ps1 = ctx.enter_context(tc.tile_pool(name="ps1", bufs=1, space="PSUM"))
        dram = ctx.enter_context(tc.tile_pool(name="dram", bufs=1, space="DRAM"))

        # ---- persistent small tiles
        w1t = sb.tile([P, kch * HID], f32, tag="w1")
        nc.sync.dma_start(
            w1t[:].rearrange("p (k h) -> p k h", h=HID),
            io["W1"].rearrange("(k p) h -> p k h", p=P),
        )
        w2t = sb.tile([HID, NCLS], f32, tag="w2")
        nc.sync.dma_start(w2t[:], io["W2"])

        ones1 = sb.tile([1, P], f32, tag="ones1")
        nc.vector.memset(ones1[:], 1.0)
        b1s = sb.tile([1, HID], f32, tag="b1s")
        nc.sync.dma_start(b1s[:], io["b1"])
        b2s = sb.tile([1, NCLS], f32, tag="b2s")
        nc.sync.dma_start(b2s[:], io["b2"])
        b1p = ps1.tile([P, HID], f32, tag="biasp")
        nc.tensor.matmul(b1p[:], lhsT=ones1[:], rhs=b1s[:], start=True, stop=True)
        b1bc = sb.tile([P, HID], f32, tag="b1bc")
        nc.vector.tensor_copy(b1bc[:], b1p[:])
        b2p = ps1.tile([P, NCLS], f32, tag="biasp")
        nc.tensor.matmul(b2p[:], lhsT=ones1[:], rhs=b2s[:], start=True, stop=True)
        b2bc = sb.tile([P, NCLS], f32, tag="b2bc")
        nc.vector.tensor_copy(b2bc[:], b2p[:])

        zrow = sb.tile([P, EP], f32, tag="zrow")
        nc.vector.memset(zrow[:], 0.0)

        dn0 = sb.tile([P, tslot], f32, tag="dn0")
        nc.sync.dma_start(dn0[:], io["deg_node"])
        dn1 = sb.tile([P, tslot], f32, tag="dn1")
        nc.vector.reciprocal(dn1[:], dn0[:])
        disn = sb.tile([P, tslot], f32, tag="disn")
        nc.scalar.activation(disn[:], dn1[:], AF.Sqrt)
        disn3h = disn[:].unsqueeze(2).to_broadcast([P, tslot, HID])

        ident = sb.tile([P, P], f32, tag="ident")
        from concourse.masks import make_identity
        make_identity(nc, ident[:])

        # merge index tiles (persistent, shared by both layers)
        mits = []
        for q in range(NCHUNK):
            mit = sb.tile([P, padloc // 16], i16, tag=f"mit{q}")
            nc.sync.dma_start(mit[:], io["midx"][q])
            mits.append(mit)

        for _rep in range(reps):
            # ---- phase A: h_hat = dis_node * (x @ W1)   (node-tile layout)
            hh = sb.tile([P, tslot * HID], f32, tag="hh")
            nc.vector.memset(hh[:], 0.0)
            for t in ([] if "noA" in ABLATE else range(tslot)):
                w = min(P, nloc - t * P)
                xt = xb.tile([P, kch * P], f32, tag="xt")
                nc.sync.dma_start(
                    xt[:, : kch * w].rearrange("p (k n) -> p k n", k=kch),
                    io["xT"][:, t * P: t * P + w].rearrange("(k p) n -> p k n", p=P),
                )
                hp = ps.tile([P, HID], f32, tag="hp")
                for k in range(kch):
                    nc.tensor.matmul(
                        hp[:w, :],
                        lhsT=xt[:, k * w: (k + 1) * w],
                        rhs=w1t[:, k * HID: (k + 1) * HID],
                        start=(k == 0),
                        stop=(k == kch - 1),
                    )
                nc.vector.tensor_scalar_mul(
                    hh[:w, t * HID: (t + 1) * HID], hp[:w, :], disn[:w, t: t + 1]
                )

            def dump_table(src16, dst_dram):
                """[128, tslot*16] SBUF -> [padloc, EP] DRAM (256B pitch)."""
                nc.sync.dma_start(
                    dst_dram[:].rearrange("(t p) e -> p t e", p=P)[:, :, :HID],
                    src16[:].rearrange("p (t h) -> p t h", h=HID),
                )
                nc.sync.dma_start(dst_dram[nloc:padloc, :], zrow[: padloc - nloc, :])

            def allgather(local_dram, table_dram):
                if "noag" in ABLATE:
                    nc.sync.dma_start(table_dram[0: padloc, :], local_dram[:])
                    return
                nc.gpsimd.collective_compute(
                    "AllGather", AL.bypass,
                    replica_groups=[list(range(NCORES))],
                    ins=[local_dram[:].opt()], outs=[table_dram[:].opt()],
                )

            hhd = dram.tile([padloc, EP], f32, tag="hhd")
            table1 = dram.tile([NCORES * padloc, EP], f32, tag="table1",
                               addr_space="Shared")
            table2 = dram.tile([NCORES * padloc, EP], f32, tag="table2",
                               addr_space="Shared")
            ztd = dram.tile([padloc, EP], f32, tag="ztd")
            accd = [dram.tile([padloc, EP], f32, tag=f"accd{q}", name=f"accd{q}")
                    for q in range(NCHUNK)]

            # ---- gather + per-chunk segment sums + merge
            def propagate(table_dram, self16, layer):
                accs = []
                for q in range(NCHUNK):
                    a = tp.tile([P, tslot * HID], f32, tag=f"acc{q}")
                    nc.vector.memset(a[:], 0.0)
                    accs.append(a)
                for b, (q, rl) in enumerate([] if "nogather" in ABLATE
                                            else batches):
                    used = rl[-1][0] + (-(-rl[-1][1] // P))
                    it = ib.tile([P, s_slots * 8], i16, tag="it")
                    nc.sync.dma_start(it[:], io["gidx"][b])
                    st = stg.tile([P, s_slots * EP], f32, tag="st")
                    st3 = st[:].rearrange("p (s e) -> p s e", e=EP)
                    nc.gpsimd.dma_gather(
                        out_ap=st3[:, :used, :],
                        in_ap=table_dram[q * crows: (q + 1) * crows, :],
                        idxs_ap=it[:, : used * 8],
                        num_idxs=used * P,
                        num_idxs_reg=used * P,
                        elem_size=EP,
                        single_packet=False,
                    )
                    a3 = accs[q][:].rearrange("p (t h) -> p t h", h=HID)
                    for (o, n) in rl:
                        tf, rem = n // P, n % P
                        if tf:
                            nc.vector.tensor_add(
                                a3[:, :tf, :], a3[:, :tf, :],
                                st3[:, o: o + tf, :HID])
                        if rem:
                            nc.vector.tensor_add(
                                a3[:rem, tf: tf + 1, :], a3[:rem, tf: tf + 1, :],
                                st3[:rem, o + tf: o + tf + 1, :HID])
                # merge back to node order: tot = self + sum_q gather(acc_q)
                tot = sb.tile([P, tslot * HID], f32, tag=f"tot{layer}")
                nc.vector.tensor_copy(tot[:], self16[:])
                tot3 = tot[:].rearrange("p (t h) -> p t h", h=HID)
                for q in ([] if "nomerge" in ABLATE else range(NCHUNK)):
                    dump_q = accd[q]
                    nc.sync.dma_start(
                        dump_q[:].rearrange("(t p) e -> p t e", p=P)[:, :, :HID],
                        accs[q][:].rearrange("p (t h) -> p t h", h=HID),
                    )
                    mst = stg.tile([P, s_slots * EP], f32, tag="st")
                    mst3 = mst[:].rearrange("p (s e) -> p s e", e=EP)
                    nc.gpsimd.dma_gather(
                        out_ap=mst3[:, : padloc // P, :],
                        in_ap=dump_q[:],
                        idxs_ap=mits[q][:],
                        num_idxs=padloc,
                        num_idxs_reg=padloc,
                        elem_size=EP,
                        single_packet=False,
                    )
                    nc.vector.tensor_add(
                        tot3, tot3, mst3[:, : padloc // P, :HID])
                return tot

            dump_table(hh, hhd)
            allgather(hhd, table1)
            tot1 = propagate(table1, hh, 1)

            # ---- z_hat = dis * relu(dis * tot1 + b1)   (node order)
            zt = sb.tile([P, tslot * HID], f32, tag="zt")
            zt3 = zt[:].rearrange("p (t h) -> p t h", h=HID)
            tot13 = tot1[:].rearrange("p (t h) -> p t h", h=HID)
            nc.vector.tensor_tensor(zt3, tot13, disn3h, op=AL.mult)
            nc.vector.tensor_tensor(
                zt3, zt3, b1bc[:].unsqueeze(1).to_broadcast([P, tslot, HID]),
                op=AL.add)
            nc.scalar.activation(zt[:], zt[:], AF.Relu)
            nc.vector.tensor_tensor(zt3, zt3, disn3h, op=AL.mult)

            dump_table(zt, ztd)
            allgather(ztd, table2)

            tot2 = propagate(table2, zt, 2)

            # ---- p = dis * tot2 ; logits = p @ W2 + b2 ; log_softmax
            pf = sb.tile([P, tslot * HID], f32, tag="pf")
            pf3 = pf[:].rearrange("p (t h) -> p t h", h=HID)
            nc.vector.tensor_tensor(
                pf3, tot2[:].rearrange("p (t h) -> p t h", h=HID), disn3h,
                op=AL.mult)

            lg = sb.tile([P, tslot * NCLS], f32, tag="lg")
            for t in range(tslot):
                ptp = ps.tile([HID, P], f32, tag="ptp")
                nc.tensor.transpose(ptp[:], pf[:, t * HID: (t + 1) * HID], ident[:])
                pts = tp.tile([HID, P], f32, tag="pts")
                nc.vector.tensor_copy(pts[:], ptp[:])
                lp = ps.tile([P, NCLS], f32, tag="lp")
                nc.tensor.matmul(lp[:], lhsT=pts[:], rhs=w2t[:], start=True, stop=True)
                nc.vector.tensor_add(lg[:, t * NCLS: (t + 1) * NCLS], lp[:], b2bc[:])

            lg3 = lg[:].rearrange("p (t c) -> p t c", c=NCLS)
            mx = sb.tile([P, tslot], f32, tag="mx")
            nc.vector.reduce_max(out=mx[:], in_=lg3, axis=mybir.AxisListType.X)
            nc.vector.tensor_tensor(
                lg3, lg3, mx[:].unsqueeze(2).to_broadcast([P, tslot, NCLS]),
                op=AL.subtract)
            ex = sb.tile([P, tslot * NCLS], f32, tag="ex")
            nc.scalar.activation(ex[:], lg[:], AF.Exp)
            sm = sb.tile([P, tslot], f32, tag="sm")
            nc.vector.reduce_sum(
                out=sm[:], in_=ex[:].rearrange("p (t c) -> p t c", c=NCLS),
                axis=mybir.AxisListType.X)
            ls = sb.tile([P, tslot], f32, tag="ls")
            nc.scalar.activation(ls[:], sm[:], AF.Ln)
            nc.vector.tensor_tensor(
                lg3, lg3, ls[:].unsqueeze(2).to_broadcast([P, tslot, NCLS]),
                op=AL.subtract)
            nc.sync.dma_start(io["out_raw"], lg[:])


def build_nc(meta, reps=1):
    import concourse.bacc as bacc
    import concourse.tile as tile
    from concourse import mybir

    nloc, tslot, padloc = _dims()
    f32, i16 = mybir.dt.float32, mybir.dt.int16
    ng, s_slots = meta["ng"], meta["s_slots"]

    nc = bacc.Bacc("TRN2", target_bir_lowering=False, debug=False,
                   num_devices=NCORES)
    io = {
        "xT": nc.dram_tensor("xT", [F_IN, nloc], f32, kind="ExternalInput").ap(),
        "W1": nc.dram_tensor("W1", [F_IN, HID], f32, kind="ExternalInput").ap(),
        "b1": nc.dram_tensor("b1", [1, HID], f32, kind="ExternalInput").ap(),
        "W2": nc.dram_tensor("W2", [HID, NCLS], f32, kind="ExternalInput").ap(),
        "b2": nc.dram_tensor("b2", [1, NCLS], f32, kind="ExternalInput").ap(),
        "deg_node": nc.dram_tensor("deg_node", [P, tslot], f32,
                                   kind="ExternalInput").ap(),
        "gidx": nc.dram_tensor("gidx", [ng, P, s_slots * 8], i16,
                               kind="ExternalInput").ap(),
        "midx": nc.dram_tensor("midx", [NCHUNK, P, padloc // 16], i16,
                               kind="ExternalInput").ap(),
        "out_raw": nc.dram_tensor("out_raw", [P, tslot * NCLS], f32,
                                  kind="ExternalOutput").ap(),
    }
    with tile.TileContext(nc) as tc:
        _emit(tc, io, meta, reps=reps)
    nc.compile()
    return nc


def make_in_maps(inputs, host):
    nloc, _, _ = _dims()
    x = np.asarray(inputs["x"], np.float32)
    W1 = np.ascontiguousarray(np.asarray(inputs["W1"], np.float32))
    b1 = np.asarray(inputs["b1"], np.float32).reshape(1, HID)
    W2 = np.ascontiguousarray(np.asarray(inputs["W2"], np.float32))
    b2 = np.asarray(inputs["b2"], np.float32).reshape(1, NCLS)
    in_maps = []
    for c in range(NCORES):
        in_maps.append({
            "xT": np.ascontiguousarray(x[c * nloc: (c + 1) * nloc].T),
            "W1": W1, "b1": b1, "W2": W2, "b2": b2,
            "deg_node": host["deg_node"][c],
            "gidx": host["gidx"][c],
            "midx": host["midx"][c],
        })
    return in_maps


def unshard(results, host):
    nloc, tslot, padloc = _dims()
    out = np.empty((N_NODES, NCLS), np.float32)
    for c in range(NCORES):
        raw = results[c]["out_raw"].reshape(P, tslot, NCLS)
        out[c * nloc: (c + 1) * nloc] = (
            raw.transpose(1, 0, 2).reshape(padloc, NCLS)[:nloc])
    return out


def run_hw_timed(nc, in_maps, iters=6):
    """Mirror of bass2jax.run_bass_via_pjrt's multi-core path, but with
    device-resident non-donated inputs so the NEFF execution can be wall-
    clock timed over repeated runs.  Returns (per-core results, best s)."""
    import time

    import jax
    from concourse import bass2jax, mybir

    bass2jax.install_neuronx_cc_hook()
    nc_mod = nc
    partition_name = (nc_mod.partition_id_tensor.name
                      if nc_mod.partition_id_tensor else None)
    in_names, out_names, out_avals, zero_outs = [], [], [], []
    for alloc in nc_mod.m.functions[0].allocations:
        if not isinstance(alloc, mybir.MemoryLocationSet):
            continue
        name = alloc.memorylocations[0].name
        if alloc.kind == "ExternalInput":
            if name != partition_name:
                in_names.append(name)
        elif alloc.kind == "ExternalOutput":
            out_names.append(name)
            shape = tuple(alloc.tensor_shape)
            dtype = mybir.dt.np(alloc.dtype)
            out_avals.append(jax.core.ShapedArray(shape, dtype))
            zero_outs.append(np.zeros(shape, dtype))
    n_params = len(in_names)
    all_names = in_names + out_names
    if partition_name is not None:
        all_names = all_names + [partition_name]

    def _body(*args):
        operands = list(args)
        if partition_name is not None:
            operands.append(bass2jax.partition_id_tensor())
        outs = bass2jax._bass_exec_p.bind(
            *operands,
            out_avals=tuple(out_avals),
            in_names=tuple(all_names),
            out_names=tuple(out_names),
            lowering_input_output_aliases=(),
            sim_require_finite=True,
            sim_require_nnan=True,
            nc=nc_mod,
        )
        return tuple(outs)

    devices = jax.devices()[:NCORES]
    mesh = bass2jax.Mesh(np.asarray(devices), ("core",))
    pspec = bass2jax.PartitionSpec("core")
    in_specs = (pspec,) * (n_params + len(out_names))
    out_specs = (pspec,) * len(out_names)
    sharded = jax.jit(
        bass2jax.shard_map(_body, mesh=mesh, in_specs=in_specs,
                           out_specs=out_specs, check_rep=False),
        keep_unused=True,
    )
    sharding = jax.sharding.NamedSharding(mesh, pspec)
    dev_in = [
        jax.device_put(
            np.concatenate([np.asarray(in_maps[c][n]) for c in range(NCORES)],
                           axis=0),
            sharding,
        )
        for n in in_names
    ]
    dev_zero = [
        jax.device_put(np.zeros((NCORES * z.shape[0], *z.shape[1:]), z.dtype),
                       sharding)
        for z in zero_outs
    ]
    jax.block_until_ready(dev_in + dev_zero)

    best = None
    out_arrs = None
    for _ in range(iters):
        t0 = time.perf_counter()
        out_arrs = sharded(*dev_in, *dev_zero)
        jax.block_until_ready(out_arrs)
        dt = time.perf_counter() - t0
        best = dt if best is None else min(best, dt)
    results = [
        {name: np.asarray(out_arrs[i]).reshape(NCORES, *out_avals[i].shape)[c]
         for i, name in enumerate(out_names)}
        for c in range(NCORES)
    ]
    return results, best


def kernel(**inputs):
    global LAST_EXEC_NS
    from concourse.bass_utils import run_bass_kernel_spmd

    meta, host = _plan(np.asarray(inputs["edge_index"]))
    nc = build_nc(meta)
    in_maps = make_in_maps(inputs, host)
    if os.environ.get("BASS_GCN_TIME", "0") == "1":
        results, best = run_hw_timed(nc, in_maps)
        LAST_EXEC_NS = int(best * 1e9)
        return unshard(results, host)
    res = run_bass_kernel_spmd(nc, in_maps, core_ids=list(range(NCORES)))
    LAST_EXEC_NS = res.exec_time_ns
    return unshard(res.results, host)

